# revision 11
# baseline (speedup 1.0000x reference)
"""GPT-NeoX attention layer (B=2, S=2048, E=2048, H=16, partial RoPE 32/128)
as a Bass/Tile kernel for 8 Trainium2 NeuronCores.

Sharding: tensor-parallel across heads (2 heads per core, Megatron-style).
Each core projects Q,K for its 2 heads (Q^T/K^T layout), projects V directly
in natural [s, d] layout (separate matmul pass, no PE transposes), applies
partial RoPE, runs causal attention, and produces a partial dense output
over its 256 columns of w_dense.  Partials are summed on the host; the dense
bias plus the (position-independent) contribution of the V bias through
w_dense are added once on the host.

Everything on device is bf16 (fp32 PSUM accumulation).  Matmul cost on the
PE is 1 cycle per moving-dim element at bf16, so the kernel is organized to
keep the PE stream dependency-free and back-to-back:
  - all SBUF pools live in one scope (separate with-scopes would serialize
    phases on SBUF reuse); only PSUM pools are phase-scoped,
  - softmax denominators come from an all-ones [128,128] stationary matmul
    (same PE cost as an M=1 ones vector, but the result lands pre-broadcast
    across partitions, so normalization is pure DVE work),
  - exp() is evaluated once per PAIR of score blocks (one activation over a
    2-bank PSUM tile) to halve the scalar-engine fixed overheads,
  - the causal mask is a 0/1 bf16 multiply applied to exp(scores) on the DVE,
  - RoPE regroups the 32 rotary rows into a [128, SF/4] layout (partition
    p = r*4+g) so the rotate-half partition swap becomes two half-tile DVE
    muls at shifted partition bases; regroup DMAs ride the Act queue, the
    writebacks ride the gpsimd queue so attention exps are never blocked,
  - dense(batch 0) tiles are interleaved into attention(batch 1); the
    dense(batch 1) tail runs in its own 4-buffer PSUM scope.
"""

import numpy as np
from contextlib import ExitStack

import concourse.bass as bass
import concourse.bacc as bacc
import concourse.mybir as mybir
import concourse.tile as tile

AF = mybir.ActivationFunctionType
F32 = mybir.dt.float32
BF16 = mybir.dt.bfloat16


class Cfg:
    def __init__(self, B=2, S=2048, E=2048, H=16, n_cores=8):
        self.B, self.S, self.E, self.H = B, S, E, H
        self.HS = 128                  # head size (fixed: one partition tile)
        self.ROT = 32                  # rotary dims
        self.n_cores = n_cores
        self.HPC = H // n_cores        # heads per core
        self.NRT = 2 * self.HPC        # q/k row tiles per core (q,k per head)
        self.RQK = self.NRT * 128      # per-core q+k rows
        self.CW = self.HPC * self.HS   # per-core v width / dense contraction
        self.CT = self.CW // 128
        self.SF = B * S                # flattened sequence
        self.KT = E // 128             # contraction tiles for projections
        self.SC = 512                  # projection column chunk
        self.NSC = self.SF // self.SC
        self.G = self.SF // 4          # rope regroup free size
        self.NCH = S // 512            # sq chunks per (b, h) pair
        self.EO = E // 128             # dense output row tiles
        self.SCALE = 1.0 / np.sqrt(self.HS)
        assert self.SF % 512 == 0 and S % 512 == 0 and E % 128 == 0


def build_program(cfg: Cfg) -> bass.Bass:
    B, S, E = cfg.B, cfg.S, cfg.E
    SF, KT, NRT, NSC, SC = cfg.SF, cfg.KT, cfg.NRT, cfg.NSC, cfg.SC
    HPC, G, NCH, EO, CT = cfg.HPC, cfg.G, cfg.NCH, cfg.EO, cfg.CT
    NSB = SF // 128                   # total s-blocks (v natural row tiles)
    SBC = SC // 128                   # s-blocks per projection chunk
    KG = 4                            # kt-group size for the first chunk

    nc = bacc.Bacc(None)
    xT = nc.dram_tensor("xT", [E, SF], BF16, kind="ExternalInput")
    wqkT = nc.dram_tensor("wqkT", [E, cfg.RQK], BF16, kind="ExternalInput")
    bqk = nc.dram_tensor("bqk", [cfg.RQK], F32, kind="ExternalInput")
    wvT = nc.dram_tensor("wvT", [E, cfg.CW], BF16, kind="ExternalInput")
    wdT = nc.dram_tensor("wdT", [cfg.CW, E], BF16, kind="ExternalInput")
    cosG = nc.dram_tensor("cosG", [128, G], BF16, kind="ExternalInput")
    sinG = nc.dram_tensor("sinG", [128, G], BF16, kind="ExternalInput")
    mask01 = nc.dram_tensor("mask01", [128, 128], BF16, kind="ExternalInput")
    outT = nc.dram_tensor("outT", [E, SF], BF16, kind="ExternalOutput")

    with tile.TileContext(nc) as tc, ExitStack() as stk:
        consts = stk.enter_context(tc.tile_pool(name="consts", bufs=1))
        qkvp = stk.enter_context(tc.tile_pool(name="qkbuf", bufs=1))
        vnatp = stk.enter_context(tc.tile_pool(name="vnat", bufs=1))
        ytp = stk.enter_context(tc.tile_pool(name="yt", bufs=1))
        wp = stk.enter_context(tc.tile_pool(name="wqk", bufs=1))
        wvp = stk.enter_context(tc.tile_pool(name="wv", bufs=1))
        wdp = stk.enter_context(tc.tile_pool(name="wd", bufs=1))
        xp = stk.enter_context(tc.tile_pool(name="xq", bufs=2))
        xvp = stk.enter_context(tc.tile_pool(name="xv", bufs=3))
        rp = stk.enter_context(tc.tile_pool(name="rope", bufs=1))
        ppool = stk.enter_context(tc.tile_pool(name="pT", bufs=3))
        npool = stk.enter_context(tc.tile_pool(name="norm", bufs=2))
        stp = stk.enter_context(tc.tile_pool(name="stage", bufs=6))

        qk_sb = qkvp.tile([128, NRT, SF], BF16)     # Q^T/K^T rows
        v_nat = vnatp.tile([128, NSB, cfg.CW], BF16)  # V natural [s, d]
        yT_sb = ytp.tile([128, HPC, SF], BF16)

        ones128 = consts.tile([128, 128], BF16)
        nc.vector.memset(ones128, 1.0)
        mask_sb = consts.tile([128, 128], BF16)
        bqk_sb = consts.tile([128, NRT], F32)
        cos_sb = consts.tile([128, G], BF16)
        sin_sb = consts.tile([128, G], BF16)

        x_view = xT.rearrange("(kt p) s -> p kt s", p=128)
        wqk_view = wqkT.rearrange("(kt p) r -> p kt r", p=128)
        wqk_sb = wp.tile([128, KT, cfg.RQK], BF16)
        wv_sb = wvp.tile([128, KT, cfg.CW], BF16)
        wd_sb = wdp.tile([128, CT, E], BF16)

        # split first-chunk weight/x loads across sync+scalar queues so the
        # PE starts within ~4us instead of waiting for two 6us transfers;
        # consts ride behind them (not needed until the first eviction)
        NKG = (KT + KG - 1) // KG
        kgs = [(kg * KG, min((kg + 1) * KG, KT)) for kg in range(NKG)]
        xt0 = xp.tile([128, KT, SC], BF16, tag="xq", name="xt0")
        for gi, (k0, k1) in enumerate(kgs):
            eng = nc.sync if gi % 2 == 0 else nc.scalar
            eng.dma_start(out=wqk_sb[:, k0:k1, :], in_=wqk_view[:, k0:k1, :])
            eng.dma_start(out=xt0[:, k0:k1, :], in_=x_view[:, k0:k1, 0:SC])
        nc.sync.dma_start(out=bqk_sb,
                          in_=bqk.rearrange("(rt p) -> p rt", p=128))
        nc.sync.dma_start(out=mask_sb, in_=mask01[:, :])
        nc.scalar.dma_start(out=cos_sb, in_=cosG[:, :])
        nc.scalar.dma_start(out=sin_sb, in_=sinG[:, :])

        # ------------- Phase 1a: Q/K projection -> qk_sb ------------------
        with tc.tile_pool(name="psqk", bufs=4, space="PSUM") as pqk, \
             tc.tile_pool(name="psv", bufs=2, space="PSUM") as pv:
            # chunk 0: kt-group-outer so matmuls start as loads land
            pss = []
            for rt in range(NRT):
                ps = pqk.tile([128, SC], F32, tag="qk", name=f"qk0_{rt}")
                pss.append(ps)
            for (k0, k1) in kgs:
                for rt in range(NRT):
                    for kt in range(k0, k1):
                        nc.tensor.matmul(
                            pss[rt], wqk_sb[:, kt, rt * 128:(rt + 1) * 128],
                            xt0[:, kt, :],
                            start=(kt == 0), stop=(kt == KT - 1),
                            skip_group_check=True)
            for rt in range(NRT):
                nc.scalar.activation(
                    qk_sb[:, rt, 0:SC], pss[rt],
                    AF.Identity, bias=bqk_sb[:, rt:rt + 1])
            # chunks 1..NSC-1: plain rt-outer
            for sc in range(1, NSC):
                xt = xp.tile([128, KT, SC], BF16, tag="xq", name=f"xt{sc}")
                nc.sync.dma_start(out=xt,
                                  in_=x_view[:, :, sc * SC:(sc + 1) * SC])
                for rt in range(NRT):
                    ps = pqk.tile([128, SC], F32, tag="qk")
                    for kt in range(KT):
                        nc.tensor.matmul(
                            ps, wqk_sb[:, kt, rt * 128:(rt + 1) * 128],
                            xt[:, kt, :],
                            start=(kt == 0), stop=(kt == KT - 1))
                    nc.scalar.activation(
                        qk_sb[:, rt, sc * SC:(sc + 1) * SC], ps,
                        AF.Identity, bias=bqk_sb[:, rt:rt + 1])

            # --------- RoPE (emitted here so DVE/Act queues run it while the
            # PE is busy with the V projection below) ----------------------
            for rt in range(NRT):
                plain = rp.tile([128, G], BF16, tag="plain")
                nc.scalar.dma_start(
                    out=plain,
                    in_=qk_sb[0:32, rt, :].rearrange("r (g c) -> r g c", g=4))
                t1 = rp.tile([128, G], BF16, tag="t1")
                t2 = rp.tile([128, G], BF16, tag="t2")
                nc.vector.tensor_mul(t2[0:64, :], plain[64:128, :],
                                     sin_sb[0:64, :])
                nc.vector.tensor_mul(t2[64:128, :], plain[0:64, :],
                                     sin_sb[64:128, :])
                nc.vector.tensor_mul(t1, plain, cos_sb)
                nc.vector.tensor_add(t1, t1, t2)
                nc.gpsimd.dma_start(
                    out=qk_sb[0:32, rt, :].rearrange("r (g c) -> r g c", g=4),
                    in_=t1)

            # --------- Phase 1b: V projection in natural [s, d] layout ----
            nc.sync.dma_start(
                out=wv_sb, in_=wvT.rearrange("(kt p) d -> p kt d", p=128))
            SCV = 256                 # finer chunks + deeper prefetch
            for sc in range(SF // SCV):
                xt = xvp.tile([128, KT, SCV], BF16, tag="xv")
                nc.sync.dma_start(out=xt,
                                  in_=x_view[:, :, sc * SCV:(sc + 1) * SCV])
                for sb in range(SCV // 128):
                    ps = pv.tile([128, cfg.CW], F32, tag="v")
                    for kt in range(KT):
                        nc.tensor.matmul(
                            ps, xt[:, kt, sb * 128:(sb + 1) * 128],
                            wv_sb[:, kt, :],
                            start=(kt == 0), stop=(kt == KT - 1))
                    nc.vector.tensor_copy(
                        v_nat[:, sc * (SCV // 128) + sb, :], ps)

        nc.sync.dma_start(
            out=wd_sb, in_=wdT.rearrange("(ct p) e -> p ct e", p=128))

        # ------------- Phase 2+3: attention + partial dense ---------------
        # Dense tiles are (scn, eo) in scn-major order so a batch's dense can
        # start as soon as its early sq-chunks finish.  Each tile is evicted
        # to a small bf16 tile and DMA'd out individually on the sync HWDGE
        # queue (x loads are done by then).
        dense_pos = {b: 0 for b in range(B)}

        def emit_dense_tiles(b, n_tiles, psd_pool, max_scn, eng_flip=False):
            done = 0
            while done < n_tiles and dense_pos[b] < EO * NCH:
                t = dense_pos[b]
                scn, eo = divmod(t, EO)
                if scn > max_scn:
                    break
                col = b * S + scn * 512
                ps = psd_pool.tile([128, 512], F32, tag="d")
                for ct in range(CT):
                    nc.tensor.matmul(
                        ps, wd_sb[:, ct, eo * 128:(eo + 1) * 128],
                        yT_sb[:, ct, col:col + 512],
                        start=(ct == 0), stop=(ct == CT - 1))
                dt = stp.tile([128, 512], BF16, tag="dt")
                if (t % 2 == 0) == eng_flip:
                    nc.vector.tensor_copy(dt, ps)
                else:
                    nc.scalar.activation(dt, ps, AF.Copy)
                nc.sync.dma_start(
                    out=outT[eo * 128:(eo + 1) * 128, col:col + 512], in_=dt)
                dense_pos[b] = t + 1
                done += 1

        DBUDGET = 6   # dense tiles interleaved per attention chunk

        with tc.tile_pool(name="psA", bufs=2, space="PSUM") as psA, \
             tc.tile_pool(name="psY", bufs=1, space="PSUM") as psY, \
             tc.tile_pool(name="psS", bufs=1, space="PSUM") as psS, \
             tc.tile_pool(name="psD", bufs=2, space="PSUM") as psD:
            for b in range(B):
                for h in range(HPC):
                    scol = b * S
                    q_t = qk_sb[:, 2 * h + 0, scol:scol + S]
                    k_t = qk_sb[:, 2 * h + 1, scol:scol + S]
                    for c in range(NCH):
                        yacc = psY.tile([128, 512], F32, tag="y")
                        sums = psS.tile([128, 512], F32, tag="s")
                        nj = 4 * c + 4
                        npair = nj // 2
                        pts = {}
                        LOOKAHEAD = 2   # score pairs in flight before accum

                        def score_pair(p):
                            ps = psA.tile([128, 2, 512], F32, tag="A",
                                          name=f"A{p}")
                            offs = []
                            for jj in range(2):
                                j = 2 * p + jj
                                off = max(0, j * 128 - c * 512)
                                offs.append(off)
                                nc.tensor.matmul(
                                    ps[:, jj, off:],
                                    k_t[:, j * 128:(j + 1) * 128],
                                    q_t[:, c * 512 + off:c * 512 + 512],
                                    start=True, stop=True,
                                    skip_group_check=True)
                            pt = ppool.tile([128, 2, 512], BF16, tag="pt",
                                            name=f"pt{p}")
                            if offs[0] == offs[1]:
                                o = offs[0]
                                nc.scalar.activation(pt[:, :, o:],
                                                     ps[:, :, o:],
                                                     AF.Exp, scale=cfg.SCALE)
                            else:  # diagonal pair: banks differ in coverage
                                for jj in range(2):
                                    o = offs[jj]
                                    nc.scalar.activation(
                                        pt[:, jj, o:], ps[:, jj, o:],
                                        AF.Exp, scale=cfg.SCALE)
                            for jj in range(2):
                                j = 2 * p + jj
                                if j >= 4 * c:  # diagonal: causal 0/1 mask
                                    off = offs[jj]
                                    nc.vector.tensor_mul(
                                        pt[:, jj, off:off + 128],
                                        pt[:, jj, off:off + 128], mask_sb)
                            pts[p] = pt

                        def accum_pair(p):
                            pt = pts.pop(p)
                            for jj in range(2):
                                j = 2 * p + jj
                                off = max(0, j * 128 - c * 512)
                                first, last = (j == 0), (j == nj - 1)
                                nc.tensor.matmul(
                                    sums[:, off:], ones128, pt[:, jj, off:],
                                    start=first, stop=last,
                                    skip_group_check=True)
                                nc.tensor.matmul(
                                    yacc[:, off:],
                                    v_nat[:, b * (S // 128) + j,
                                          h * 128:(h + 1) * 128],
                                    pt[:, jj, off:],
                                    start=first, stop=last,
                                    skip_group_check=True)

                        for p in range(npair):
                            score_pair(p)
                            if p >= LOOKAHEAD:
                                accum_pair(p - LOOKAHEAD)
                        for p in range(max(0, npair - LOOKAHEAD), npair):
                            accum_pair(p)

                        recip = npool.tile([128, 512], F32, tag="recip")
                        nc.vector.reciprocal(recip, sums)
                        nc.vector.tensor_mul(
                            yT_sb[:, h, scol + c * 512:scol + (c + 1) * 512],
                            yacc, recip)
                        # greedy dense interleave: older batches first, then
                        # this batch's finished sq-chunks (h == last only)
                        budget = DBUDGET
                        for db in range(b):
                            before = dense_pos[db]
                            emit_dense_tiles(db, budget, psD, NCH - 1,
                                             eng_flip=True)
                            budget -= dense_pos[db] - before
                        if h == HPC - 1 and budget > 0:
                            emit_dense_tiles(b, budget, psD, c - 1,
                                             eng_flip=True)

        # dense tail: own 4-deep PSUM scope
        with tc.tile_pool(name="psDt", bufs=4, space="PSUM") as psDt:
            for b in range(B):
                emit_dense_tiles(b, EO * NCH, psDt, NCH - 1)

    nc.finalize()
    return nc


# ---------------------------------------------------------------------------
# Host-side input preparation / sharding
# ---------------------------------------------------------------------------

def _bf16(a: np.ndarray) -> np.ndarray:
    import ml_dtypes
    return np.ascontiguousarray(a, np.float32).astype(ml_dtypes.bfloat16)


def _rope_tables(cfg: Cfg):
    inv_freq = 1.0 / (10000.0 ** (np.arange(0, cfg.ROT, 2, dtype=np.float64)
                                  / cfg.ROT))
    t = np.arange(cfg.S, dtype=np.float64)
    freqs = np.outer(t, inv_freq)                       # [S, 16]
    emb = np.concatenate([freqs, freqs], axis=-1)       # [S, 32]
    cos = np.cos(emb).T.astype(np.float32)              # [32, S]
    sin = np.sin(emb).T.astype(np.float32)
    cosF = np.tile(cos, (1, cfg.B))                     # [32, SF]
    sinF = np.tile(sin, (1, cfg.B))
    sinF[:cfg.ROT // 2] *= -1.0                         # fold rotate_half sign
    G = cfg.G
    # regrouped layout: partition p = r*4 + g  ->  row r, column group g
    cosR = np.ascontiguousarray(cosF.reshape(32, 4, G).reshape(128, G))
    sinR = np.ascontiguousarray(sinF.reshape(32, 4, G).reshape(128, G))
    return _bf16(cosR), _bf16(sinR)


def make_in_maps(cfg: Cfg, x, w_qkv, b_qkv, w_dense):
    HS = cfg.HS
    xTb = _bf16(x.reshape(cfg.B * cfg.S, cfg.E).T)
    cosR, sinR = _rope_tables(cfg)
    p = np.arange(128)[:, None]
    f = np.arange(128)[None, :]
    mask01 = _bf16(np.where(p <= f, 1.0, 0.0))

    bv_full = np.zeros(cfg.E, dtype=np.float64)
    in_maps = []
    for i in range(cfg.n_cores):
        qk_rows, v_rows = [], []
        for h in range(i * cfg.HPC, (i + 1) * cfg.HPC):
            base = h * 3 * HS
            qk_rows += list(range(base, base + HS))          # q rows
            qk_rows += list(range(base + HS, base + 2 * HS))  # k rows
            v_rows += list(range(base + 2 * HS, base + 3 * HS))
        qk_rows = np.array(qk_rows)
        v_rows = np.array(v_rows)
        dcols = slice(i * cfg.CW, (i + 1) * cfg.CW)
        bv_full[i * cfg.CW:(i + 1) * cfg.CW] = b_qkv[v_rows]
        in_maps.append({
            "xT": xTb,
            "wqkT": _bf16(w_qkv[qk_rows, :].T),
            "bqk": np.ascontiguousarray(b_qkv[qk_rows]).astype(np.float32),
            "wvT": _bf16(w_qkv[v_rows, :].T),
            "wdT": _bf16(w_dense[:, dcols].T),
            "cosG": cosR,
            "sinG": sinR,
            "mask01": mask01,
        })
    # position-independent V-bias contribution through the dense layer,
    # added on the host together with b_dense
    cfg._bv_dense = (np.asarray(w_dense, np.float64) @ bv_full).astype(
        np.float64)
    return in_maps


def combine_outputs(cfg: Cfg, results, b_dense):
    acc = np.zeros((cfg.E, cfg.SF), dtype=np.float32)
    for r in results:
        acc += np.asarray(r["outT"], dtype=np.float32)
    bias = np.asarray(b_dense, np.float64) + getattr(cfg, "_bv_dense", 0.0)
    out = acc.T.reshape(cfg.B, cfg.S, cfg.E).astype(np.float64) + bias
    return out.astype(np.float32)


_PROGRAM_CACHE = {}


def kernel(x, w_qkv, b_qkv, w_dense, b_dense):
    from concourse.bass_utils import run_bass_kernel_spmd

    cfg = Cfg()
    key = "full"
    if key not in _PROGRAM_CACHE:
        _PROGRAM_CACHE[key] = build_program(cfg)
    nc = _PROGRAM_CACHE[key]
    in_maps = make_in_maps(cfg, np.asarray(x), np.asarray(w_qkv),
                           np.asarray(b_qkv), np.asarray(w_dense))
    res = run_bass_kernel_spmd(nc, in_maps, list(range(cfg.n_cores)))
    return combine_outputs(cfg, res.results, np.asarray(b_dense))
